# revision 9
# baseline (speedup 1.0000x reference)
"""AlphaEntmax sparse-attention kernel for 8 Trainium2 NeuronCores.

Strategy (data-parallel over the 512 (b,c) positions, 64 per core):
  - projections q,k,v on PE (bf16, scale (alpha-1)/sqrt(E) folded into Wk)
  - per position: dot[i,j] = sum_h q[h,i] k[h,j] on PE (K=8 matmuls),
    evicted PSUM->SBUF bf16 by ACT (with row-sum accum) while DVE extracts
    row max / sumsq stats for the tau seed
  - entmax-1.5 threshold tau solved with a calibrated seed + 2 Newton
    iterations on the bf16 scores (scalar_tensor_tensor relu/square passes
    with fused per-row accumulation, split across ACT and DVE)
  - final p computed transposed: PE recomputes dot^T with two extra
    contraction rows carrying -tau (bf16 hi+lo split) so relu^2 of the PSUM
    gives p^T directly; res^T = v^T @ p^T on PE with 4-position col-grouping
  - normalization by Z = sum p (predicted from the last Newton accums) is
    folded into the res_flat^T scatter copies; final Wu matmul on PE.
"""
import numpy as np
import ml_dtypes

import concourse.bass as bass
import concourse.tile as tile
from concourse import mybir
from concourse.bass_utils import run_bass_kernel_spmd

F32 = mybir.dt.float32
BF16 = mybir.dt.bfloat16
AL = mybir.AluOpType
AF = mybir.ActivationFunctionType

N_CORES = 8
B, C, E, H = 2, 256, 512, 8
P_TOT = B * C                  # 512 positions
NPOS = P_TOT // N_CORES        # 64 per core
NROUND = 4                     # positions processed in rounds (SBUF budget)
RPOS = NPOS // NROUND          # 16 positions per round
NT_R = RPOS * 4                # score tiles per round (4 i-tiles per position)

# tau-seed calibration (fit offline on the model's input distribution):
# gap_half = mxh - tau* ~= c0 + c1*dmx + c2*sg + c3*dmx^2 + c4*sg*dmx
SEED_COEF = (0.063100, 0.970412, -0.742733, 0.211108, -1.770962)
SEED_MARGIN = 0.01

# Newton-pass engine splits (tunable): pass A on ACT every ACT_A_EVERY-th
# tile (ACT relu+accum gives t and S1 in one op; DVE path uses two 4x
# tensor_scalar ops).  pass B: DVE scalar_tensor_tensor+accum for tiles with
# (t % 8) < B_DVE_OF8, ACT Square+accum for the rest.
ACT_A_EVERY = 3
B_DVE_OF8 = 2
HALF1 = True


def _split_waits(nc, limit=1):
    """walrus here only supports one sync-wait per instruction: split extras
    into chained same-engine drains."""
    n = 0
    for fn in nc.m.functions:
        for bb in fn.blocks:
            newlist = []
            for ins in bb.instructions:
                si = getattr(ins, "sync_info", None)
                if si is not None and len(si.on_wait) > limit:
                    waits = list(si.on_wait)
                    while len(waits) > limit:
                        chunk, waits = waits[:limit], waits[limit:]
                        d = mybir.InstDrain(name=f"Wsplit-{n}")
                        n += 1
                        d.engine = ins.engine
                        d.sync_info = mybir.SyncInfo(on_wait=list(chunk), on_update=[])
                        newlist.append(d)
                    del si.on_wait[:]
                    si.on_wait.extend(waits)
                newlist.append(ins)
            del bb.instructions[:]
            for x in newlist:
                bb.add_instruction(x)
    return n


def build_nc():
    nc = bass.Bass("TRN2", target_bir_lowering=False, debug=False)

    xT = nc.declare_dram_parameter("xT", [E, NPOS], BF16, isOutput=False)
    wqk = nc.declare_dram_parameter("wqk", [E, 2 * E * H], BF16, isOutput=False)
    bqk = nc.declare_dram_parameter("bqk", [1, 2 * E * H], BF16, isOutput=False)
    wv = nc.declare_dram_parameter("wv", [E, E * H], BF16, isOutput=False)
    bv = nc.declare_dram_parameter("bv", [1, E * H], BF16, isOutput=False)
    wu = nc.declare_dram_parameter("wu", [E * H, E], BF16, isOutput=False)
    bu = nc.declare_dram_parameter("bu", [1, E], BF16, isOutput=False)
    out = nc.declare_dram_parameter("out", [NPOS, E], F32, isOutput=True)

    with tile.TileContext(nc) as tc:
        _build_body(nc, tc, xT, wqk, bqk, wv, bv, wu, bu, out)
    _split_waits(nc)
    return nc


def _build_body(nc, tc, xT, wqk, bqk, wv, bv, wu, bu, out):
    from contextlib import ExitStack
    ctx = ExitStack()
    sing = ctx.enter_context(tc.tile_pool(name="sing", bufs=1))
    wstream = ctx.enter_context(tc.tile_pool(name="wstream", bufs=2))
    evj = ctx.enter_context(tc.tile_pool(name="evj", bufs=6))
    tbufs = ctx.enter_context(tc.tile_pool(name="tbufs", bufs=6))
    stats = ctx.enter_context(tc.tile_pool(name="stats", bufs=1))
    final = ctx.enter_context(tc.tile_pool(name="final", bufs=2))

    # ---- persistent tiles ----
    s_xT = sing.tile([128, 4, NPOS], BF16)          # xT Ktiles
    nc.sync.dma_start(out=s_xT, in_=xT.rearrange("(kt p) n -> p kt n", p=128))
    s_bqk = sing.tile([1, 2 * E * H], BF16)
    nc.sync.dma_start(out=s_bqk, in_=bqk[:, :])
    s_bv = sing.tile([1, E * H], BF16)
    nc.sync.dma_start(out=s_bv, in_=bv[:, :])
    s_bu = sing.tile([1, E], BF16)
    nc.sync.dma_start(out=s_bu, in_=bu[:, :])
    s_ones = sing.tile([1, 512], BF16)
    nc.vector.memset(s_ones, 1.0)
    s_zero = sing.tile([128, 512], BF16)
    nc.vector.memset(s_zero, 0.0)

    q_pack = sing.tile([128, NPOS // 4, 512], BF16)   # rows 32g+h ; +8,+9 = -tau rows
    k_pack = sing.tile([128, NPOS // 4, 512], BF16)   # rows 32g+8/9 = ones
    nc.vector.memset(k_pack[:, :, :], 1.0)   # rows 32g+8/9 stay 1.0; k DMAs overwrite 0..7
    vT_pack = sing.tile([128, NPOS, 4, 8], BF16)      # [j_lo, pos, j_hi, h]
    resf = sing.tile([128, 8, 4, NPOS], BF16)         # res_flat^T [(i_lo),(h,i_hi,pos)]
    stg_ctx = tc.tile_pool(name="stg", bufs=1)
    stg = stg_ctx.__enter__()
    q_stage = stg.tile([NPOS, E * H], BF16)
    k_stage = stg.tile([NPOS, E * H], BF16)

    # ---- projections: q, k  (out[pos, hi] = xT.T @ W + b) ----
    pp_ctx = tc.tile_pool(name="pp", bufs=2, space="PSUM")
    pp = pp_ctx.__enter__()
    for c in range(16):
        wchunk = wstream.tile([128, 4, 512], BF16, tag="wqk")
        nc.sync.dma_start(
            out=wchunk,
            in_=wqk[:, 512 * c: 512 * (c + 1)].rearrange("(kt p) n -> p kt n", p=128))
        ps = pp.tile([NPOS, 512], F32, tag="psproj")
        for kt in range(4):
            nc.tensor.matmul(ps, s_xT[:, kt, :], wchunk[:, kt, :],
                             start=(kt == 0), stop=False)
        nc.tensor.matmul(ps, s_ones[:, :NPOS], s_bqk[:, 512 * c: 512 * (c + 1)],
                         start=False, stop=True)
        stage = q_stage if c < 8 else k_stage
        cc = c % 8
        nc.vector.tensor_scalar(stage[:, 512 * cc: 512 * (cc + 1)], ps, 1.0, None,
                                AL.mult, AL.bypass)
    # repack q/k into [h + 32*(p%4), p//4, e]
    for p in range(NPOS):
        g, pg = p % 4, p // 4
        nc.sync.dma_start(
            out=q_pack[32 * g: 32 * g + 8, pg, :],
            in_=q_stage[p: p + 1, :].rearrange("p (h e) -> p h e", h=8))
        nc.sync.dma_start(
            out=k_pack[32 * g: 32 * g + 8, pg, :],
            in_=k_stage[p: p + 1, :].rearrange("p (h e) -> p h e", h=8))

    stg_ctx.__exit__(None, None, None)

    # ---- projection v (transposed orientation): out[hj, pos] = Wv.T @ x ----
    for c in range(8):
        wvchunk = wstream.tile([128, 4, 512], BF16, tag="wv")
        nc.sync.dma_start(
            out=wvchunk,
            in_=wv[:, 512 * c: 512 * (c + 1)].rearrange("(kt p) n -> p kt n", p=128))
        for sub in range(4):
            t = 4 * c + sub          # hj-tile index: h = t // 4, j_hi = t % 4
            h, j_hi = t // 4, t % 4
            psv = pp.tile([128, NPOS], F32, tag="psv")
            for kt in range(4):
                nc.tensor.matmul(psv, wvchunk[:, kt, 128 * sub: 128 * (sub + 1)],
                                 s_xT[:, kt, :], start=(kt == 0), stop=False)
            nc.tensor.matmul(psv, s_bv[:, 128 * t: 128 * (t + 1)],
                             s_ones[:, :NPOS], start=False, stop=True)
            nc.vector.tensor_scalar(vT_pack[:, :, j_hi, h], psv, 1.0, None,
                                    AL.mult, AL.bypass)
    pp_ctx.__exit__(None, None, None)

    # ---- per-round processing ----
    pd_ctx = tc.tile_pool(name="pd", bufs=4, space="PSUM")
    pd = pd_ctx.__enter__()
    pf_ctx = tc.tile_pool(name="pf", bufs=1, space="PSUM")
    pf = pf_ctx.__enter__()
    pt_ctx = tc.tile_pool(name="ptp", bufs=2, space="PSUM")
    ptp = pt_ctx.__enter__()
    for r in range(NROUND):
        p0 = r * RPOS
        Xa = stats.tile([128, NT_R, 512], BF16, tag="xa")
        musum = stats.tile([128, NT_R], F32, tag="musum")
        mxh = stats.tile([128, NT_R], F32, tag="mxh")
        sqq = stats.tile([128, NT_R], F32, tag="sqq")

        # stage B: dot + evict + stats
        for pl in range(RPOS):
            p = p0 + pl
            g, pg = p % 4, p // 4
            for it in range(4):
                t = pl * 4 + it
                psd = pd.tile([128, 512], F32, tag="psd")
                nc.tensor.matmul(psd, q_pack[32 * g: 32 * g + 8, pg,
                                             128 * it: 128 * (it + 1)],
                                 k_pack[32 * g: 32 * g + 8, pg, :],
                                 start=True, stop=True,
                                 tile_position=(32 * g, 0))
                nc.scalar.activation(Xa[:, t, :], psd, AF.Copy, bias=0.0,
                                     scale=1.0, accum_out=musum[:, t: t + 1])
                ej = evj.tile([128, 256], BF16, tag="ej")
                nc.vector.tensor_scalar(ej, psd[:, 0:256], 1.0, None,
                                        AL.mult, AL.max,
                                        accum_out=mxh[:, t: t + 1])
        # sumsq (quarter width) from evicted bf16 scores
        for t in range(NT_R):
            ej2 = evj.tile([128, 128], BF16, tag="ej2")
            nc.vector.scalar_tensor_tensor(ej2, Xa[:, t, 0:128], 1.0,
                                           Xa[:, t, 0:128], AL.bypass, AL.mult,
                                           accum_out=sqq[:, t: t + 1])

        # ---- seed ----
        mu = stats.tile([128, NT_R], F32, tag="mu")
        nc.vector.tensor_scalar(mu, musum, 1.0 / 512.0, None, AL.mult, AL.bypass)
        dmx = stats.tile([128, NT_R], F32, tag="dmx")
        nc.vector.tensor_tensor(dmx, mxh, mu, AL.subtract)
        var = stats.tile([128, NT_R], F32, tag="var")
        # var = sqq/128 - mu^2   (clamped)
        nc.vector.scalar_tensor_tensor(var, mu, 0.0, mu, AL.mult, AL.mult)  # mu^2
        nc.vector.scalar_tensor_tensor(var, sqq, 1.0 / 128.0, var,
                                       AL.mult, AL.subtract)
        nc.vector.tensor_scalar(var, var, 1e-8, None, AL.max, AL.bypass)
        sg = stats.tile([128, NT_R], F32, tag="sg")
        nc.scalar.activation(sg, var, AF.Sqrt, bias=0.0, scale=1.0)
        # pred = c0+margin + c1*dmx + c2*sg + c3*dmx^2 + c4*sg*dmx
        c0, c1, c2, c3, c4 = SEED_COEF
        pred = stats.tile([128, NT_R], F32, tag="pred")
        tmp = stats.tile([128, NT_R], F32, tag="tmp")
        nc.vector.tensor_scalar(pred, dmx, c1, c0 + SEED_MARGIN, AL.mult, AL.add)
        nc.vector.tensor_scalar(tmp, sg, c2, None, AL.mult, AL.bypass)
        nc.vector.tensor_tensor(pred, pred, tmp, AL.add)
        nc.vector.scalar_tensor_tensor(tmp, dmx, c3, dmx, AL.mult, AL.mult)
        nc.vector.tensor_tensor(pred, pred, tmp, AL.add)
        nc.vector.scalar_tensor_tensor(tmp, sg, c4, dmx, AL.mult, AL.mult)
        nc.vector.tensor_tensor(pred, pred, tmp, AL.add)
        # tau = max(mxh - pred, mxh - 1)
        tau = stats.tile([128, NT_R], F32, tag="tau")
        nc.vector.tensor_tensor(tau, mxh, pred, AL.subtract)
        nc.vector.tensor_scalar(tmp, mxh, -1.0, None, AL.add, AL.bypass)
        nc.vector.tensor_tensor(tau, tau, tmp, AL.max)
        ntau = stats.tile([128, NT_R], F32, tag="ntau")
        S1 = stats.tile([128, NT_R], F32, tag="S1")
        S2 = stats.tile([128, NT_R], F32, tag="S2")
        dlt = stats.tile([128, NT_R], F32, tag="dlt")

        # ---- 2 Newton iterations ----
        for itn in range(2):
            half = HALF1 and itn == 0
            W = 256 if half else 512
            tgt = -0.5 if half else -1.0
            nc.vector.tensor_scalar(ntau, tau, -1.0, None, AL.mult, AL.bypass)
            for t in range(NT_R):
                tb = tbufs.tile([128, 512], BF16, tag="tb")
                if t % ACT_A_EVERY == 0:
                    nc.scalar.activation(tb[:, :W], Xa[:, t, :W], AF.Relu,
                                         bias=ntau[:, t: t + 1], scale=1.0,
                                         accum_out=S1[:, t: t + 1])
                else:
                    nc.vector.scalar_tensor_tensor(tb[:, :W], Xa[:, t, :W],
                                                   tau[:, t: t + 1],
                                                   s_zero[:, :W],
                                                   AL.subtract, AL.max,
                                                   accum_out=S1[:, t: t + 1])
                ej3 = evj.tile([128, 512], BF16, tag="ej3")
                if t % 8 < B_DVE_OF8:
                    nc.vector.scalar_tensor_tensor(ej3[:, :W], tb[:, :W], 1.0,
                                                   tb[:, :W],
                                                   AL.bypass, AL.mult,
                                                   accum_out=S2[:, t: t + 1])
                else:
                    nc.scalar.activation(ej3[:, :W], tb[:, :W], AF.Square,
                                         bias=0.0, scale=1.0,
                                         accum_out=S2[:, t: t + 1])
            # delta = (S2 + tgt) * 0.5 / S1 ;  tau += delta
            rec = stats.tile([128, NT_R], F32, tag="rec")
            nc.vector.reciprocal(rec, S1)
            nc.vector.tensor_scalar(dlt, S2, tgt, 0.5, AL.add, AL.mult)
            nc.vector.tensor_tensor(dlt, dlt, rec, AL.mult)
            nc.vector.tensor_tensor(tau, tau, dlt, AL.add)

        # Z = S2 - 2*delta*S1 ; Zinv
        Z = stats.tile([128, NT_R], F32, tag="Z")
        nc.vector.tensor_tensor(Z, dlt, S1, AL.mult)
        nc.vector.scalar_tensor_tensor(Z, Z, -2.0, S2, AL.mult, AL.add)
        nc.vector.tensor_scalar(Z, Z, 1e-30, None, AL.max, AL.bypass)
        Zinv = stats.tile([128, NT_R], F32, tag="Zinv")
        nc.vector.reciprocal(Zinv, Z)

        # ---- tau split (bf16 hi + residual) + transpose into q_pack rows ----
        nc.vector.tensor_scalar(ntau, tau, -1.0, None, AL.mult, AL.bypass)
        nth_b = stats.tile([128, NT_R], BF16, tag="nth_b")
        nc.vector.tensor_scalar(nth_b, ntau, 1.0, None, AL.mult, AL.bypass)
        nth_f = stats.tile([128, NT_R], F32, tag="nth_f")
        nc.vector.tensor_scalar(nth_f, nth_b, 1.0, None, AL.mult, AL.bypass)
        nr_b = stats.tile([128, NT_R], BF16, tag="nr_b")
        nc.vector.tensor_tensor(ntau, ntau, nth_f, AL.subtract)   # residual (f32)
        nc.vector.tensor_scalar(nr_b, ntau, 1.0, None, AL.mult, AL.bypass)
        tauI = stats.tile([128, 8 * RPOS], BF16, tag="tauI")
        nc.vector.tensor_scalar(
            tauI.rearrange("q (p it c) -> q p it c", it=4, c=2)[:, :, :, 0],
            nth_b.rearrange("q (p it) -> q p it", it=4), 1.0, None,
            AL.mult, AL.bypass)
        nc.vector.tensor_scalar(
            tauI.rearrange("q (p it c) -> q p it c", it=4, c=2)[:, :, :, 1],
            nr_b.rearrange("q (p it) -> q p it", it=4), 1.0, None,
            AL.mult, AL.bypass)
        tauIT = stats.tile([128, 8 * RPOS], BF16, tag="tauIT")
        for blk in range(RPOS // 16):
            nc.sync.dma_start(out=tauIT[:, 128 * blk: 128 * (blk + 1)],
                              in_=tauI[:, 128 * blk: 128 * (blk + 1)],
                              transpose=True)
        tauIT_v = tauIT.rearrange("q (blk f) -> q blk f", f=128)
        for pl in range(RPOS):
            p = p0 + pl
            g, pg = p % 4, p // 4
            blk, row = pl // 16, (pl % 16) * 8
            nc.sync.dma_start(
                out=q_pack[32 * g + 8: 32 * g + 9, pg, :]
                    .rearrange("p (a b) -> p a b", a=4),
                in_=tauIT_v[row: row + 8: 2, blk, :])
            nc.sync.dma_start(
                out=q_pack[32 * g + 9: 32 * g + 10, pg, :]
                    .rearrange("p (a b) -> p a b", a=4),
                in_=tauIT_v[row + 1: row + 8: 2, blk, :])

        # ---- final phase: dot^T - tau, p^T, res^T, normalize+scatter ----
        for q4 in range(RPOS // 4):        # groups of 4 positions
            psr = pf.tile([128, 512], F32, tag="psr")
            pTq = [None] * 4
            for gl in range(4):
                pl = q4 * 4 + gl
                p = p0 + pl
                g, pg = p % 4, p // 4
                pT = final.tile([128, 4, 512], BF16, tag="pt")
                pTq[gl] = pT
                for jt in range(4):
                    psT = ptp.tile([128, 512], F32, tag="pst")
                    nc.tensor.matmul(psT,
                                     k_pack[32 * g: 32 * g + 10, pg,
                                            128 * jt: 128 * (jt + 1)],
                                     q_pack[32 * g: 32 * g + 10, pg, :],
                                     start=True, stop=True,
                                     tile_position=(32 * g, 0))
                    tb2 = tbufs.tile([128, 512], BF16, tag="tb2")
                    nc.scalar.activation(tb2, psT, AF.Relu, bias=0.0,
                                         scale=1.0)
                    if jt % 2 == 0:
                        nc.vector.scalar_tensor_tensor(pT[:, jt, :], tb2, 1.0,
                                                       tb2, AL.bypass, AL.mult)
                    else:
                        nc.scalar.activation(pT[:, jt, :], tb2, AF.Square,
                                             bias=0.0, scale=1.0)
                # res^T for this position into col-group 32*gl of psr
                for jt in range(4):
                    nc.tensor.matmul(psr[32 * gl: 32 * gl + 8, :],
                                     vT_pack[:, p, jt, :], pTq[gl][:, jt, :],
                                     start=(jt == 0), stop=(jt == 3),
                                     tile_position=(0, 32 * gl))
            resN = final.tile([128, 512], BF16, tag="resn")
            nc.scalar.activation(resN, psr, AF.Copy, bias=0.0, scale=1.0)
            for i_hi in range(4):
                rT = final.tile([128, 128], BF16, tag="rt")
                nc.sync.dma_start(out=rT, in_=resN[:, 128 * i_hi: 128 * (i_hi + 1)],
                                  transpose=True)
                for gl in range(4):
                    pl = q4 * 4 + gl
                    p = p0 + pl
                    t = pl * 4 + i_hi
                    dst = resf[:, :, i_hi, p]       # [128, 8] strided (h)
                    if gl % 2 == 0:
                        nc.scalar.activation(dst, rT[:, 32 * gl: 32 * gl + 8],
                                             AF.Copy, bias=0.0,
                                             scale=Zinv[:, t: t + 1])
                    else:
                        nc.vector.tensor_scalar(dst, rT[:, 32 * gl: 32 * gl + 8],
                                                Zinv[:, t: t + 1], None,
                                                AL.mult, AL.bypass)

    pt_ctx.__exit__(None, None, None)
    pf_ctx.__exit__(None, None, None)
    pd_ctx.__exit__(None, None, None)

    # ---- output projection ----
    po_ctx = tc.tile_pool(name="po", bufs=1, space="PSUM")
    po = po_ctx.__enter__()
    pso = po.tile([NPOS, 512], F32, tag="pso")
    for t in range(32):
        h, i_hi = t // 4, t % 4
        wuk = wstream.tile([128, 512], BF16, tag="wu")
        nc.sync.dma_start(out=wuk, in_=wu[128 * t: 128 * (t + 1), :])
        nc.tensor.matmul(pso, resf[:, h, i_hi, :], wuk,
                         start=(t == 0), stop=False)
    nc.tensor.matmul(pso, s_ones[:, :NPOS], s_bu, start=False, stop=True)
    out_sb = sing.tile([NPOS, 512], F32)
    nc.scalar.activation(out_sb, pso, AF.Copy, bias=0.0, scale=1.0)
    nc.sync.dma_start(out=out[:, :], in_=out_sb)
    po_ctx.__exit__(None, None, None)
    ctx.close()


_NC_CACHE = {}


def _get_nc():
    if "nc" not in _NC_CACHE:
        _NC_CACHE["nc"] = build_nc()
    return _NC_CACHE["nc"]


def kernel(x, Wq, bq, Wk, bk, Wv, bv, Wu, bu, alpha):
    alpha = float(np.asarray(alpha))
    assert abs(alpha - 1.5) < 1e-6, "kernel specialized for alpha=1.5"
    s = np.float32((alpha - 1.0) / np.sqrt(E))
    bf = ml_dtypes.bfloat16
    x2 = np.asarray(x, np.float32).reshape(P_TOT, E)
    wqk_h = np.concatenate([np.asarray(Wq, np.float32),
                            np.asarray(Wk, np.float32) * s], axis=1).astype(bf)
    bqk_h = np.concatenate([np.asarray(bq, np.float32),
                            np.asarray(bk, np.float32) * s])[None, :].astype(bf)
    wv_h = np.asarray(Wv, np.float32).astype(bf)
    bv_h = np.asarray(bv, np.float32)[None, :].astype(bf)
    wu_h = np.asarray(Wu, np.float32).astype(bf)
    bu_h = np.asarray(bu, np.float32)[None, :].astype(bf)

    nc = _get_nc()
    in_maps = []
    for core in range(N_CORES):
        sh = x2[core * NPOS:(core + 1) * NPOS, :]     # [NPOS, E]
        in_maps.append(dict(
            xT=np.ascontiguousarray(sh.T).astype(bf),
            wqk=wqk_h, bqk=bqk_h, wv=wv_h, bv=bv_h, wu=wu_h, bu=bu_h,
        ))
    r = run_bass_kernel_spmd(nc, in_maps, core_ids=list(range(N_CORES)))
    outs = [r.results[c]["out"] for c in range(N_CORES)]
    full = np.concatenate(outs, axis=0).reshape(B, C, E).astype(np.float32)
    return full


# revision 10
# speedup vs baseline: 1.1448x; 1.1448x over previous
"""AlphaEntmax sparse-attention kernel for 8 Trainium2 NeuronCores.

Strategy (data-parallel over the 512 (b,c) positions, 64 per core):
  - projections q,k,v on PE (bf16, scale (alpha-1)/sqrt(E) folded into Wk)
  - per position: dot[i,j] = sum_h q[h,i] k[h,j] on PE (K=8 matmuls),
    evicted PSUM->SBUF bf16 by ACT (with row-sum accum) while DVE extracts
    row max / sumsq stats for the tau seed
  - entmax-1.5 threshold tau solved with a calibrated seed + 2 Newton
    iterations on the bf16 scores (scalar_tensor_tensor relu/square passes
    with fused per-row accumulation, split across ACT and DVE)
  - final p computed transposed: PE recomputes dot^T with two extra
    contraction rows carrying -tau (bf16 hi+lo split) so relu^2 of the PSUM
    gives p^T directly; res^T = v^T @ p^T on PE with 4-position col-grouping
  - normalization by Z = sum p (predicted from the last Newton accums) is
    folded into the res_flat^T scatter copies; final Wu matmul on PE.
"""
import numpy as np
import ml_dtypes

import concourse.bass as bass
import concourse.tile as tile
from concourse import mybir
from concourse.bass_utils import run_bass_kernel_spmd

F32 = mybir.dt.float32
BF16 = mybir.dt.bfloat16
AL = mybir.AluOpType
AF = mybir.ActivationFunctionType

N_CORES = 8
B, C, E, H = 2, 256, 512, 8
P_TOT = B * C                  # 512 positions
NPOS = P_TOT // N_CORES        # 64 per core
NROUND = 4                     # positions processed in rounds (SBUF budget)
RPOS = NPOS // NROUND          # 16 positions per round
NT_R = RPOS * 4                # score tiles per round (4 i-tiles per position)

# tau-seed calibration (fit offline on the model's input distribution):
# gap_half = mxh - tau* ~= c0 + c1*dmx + c2*sg + c3*dmx^2 + c4*sg*dmx
SEED_COEF = (0.063100, 0.970412, -0.742733, 0.211108, -1.770962)
SEED_MARGIN = 0.01

# Newton-pass engine splits (tunable): pass A on ACT every ACT_A_EVERY-th
# tile (ACT relu+accum gives t and S1 in one op; DVE path uses two 4x
# tensor_scalar ops).  pass B: DVE scalar_tensor_tensor+accum for tiles with
# (t % 8) < B_DVE_OF8, ACT Square+accum for the rest.
ACT_A_EVERY = 8
B_DVE_OF8 = 5
HALF1 = True


def _split_waits(nc, limit=1):
    """walrus here only supports one sync-wait per instruction: split extras
    into chained same-engine drains."""
    n = 0
    for fn in nc.m.functions:
        for bb in fn.blocks:
            newlist = []
            for ins in bb.instructions:
                si = getattr(ins, "sync_info", None)
                if si is not None and len(si.on_wait) > limit:
                    waits = list(si.on_wait)
                    while len(waits) > limit:
                        chunk, waits = waits[:limit], waits[limit:]
                        d = mybir.InstDrain(name=f"Wsplit-{n}")
                        n += 1
                        d.engine = ins.engine
                        d.sync_info = mybir.SyncInfo(on_wait=list(chunk), on_update=[])
                        newlist.append(d)
                    del si.on_wait[:]
                    si.on_wait.extend(waits)
                newlist.append(ins)
            del bb.instructions[:]
            for x in newlist:
                bb.add_instruction(x)
    return n


def build_nc():
    nc = bass.Bass("TRN2", target_bir_lowering=False, debug=False)

    xT = nc.declare_dram_parameter("xT", [E, NPOS], BF16, isOutput=False)
    wqk = nc.declare_dram_parameter("wqk", [E, 2 * E * H], BF16, isOutput=False)
    bqk = nc.declare_dram_parameter("bqk", [1, 2 * E * H], BF16, isOutput=False)
    wv = nc.declare_dram_parameter("wv", [E, E * H], BF16, isOutput=False)
    bv = nc.declare_dram_parameter("bv", [1, E * H], BF16, isOutput=False)
    wu = nc.declare_dram_parameter("wu", [E * H, E], BF16, isOutput=False)
    bu = nc.declare_dram_parameter("bu", [1, E], BF16, isOutput=False)
    out = nc.declare_dram_parameter("out", [NPOS, E], F32, isOutput=True)

    with tile.TileContext(nc) as tc:
        _build_body(nc, tc, xT, wqk, bqk, wv, bv, wu, bu, out)
    _split_waits(nc)
    return nc


def _build_body(nc, tc, xT, wqk, bqk, wv, bv, wu, bu, out):
    from contextlib import ExitStack
    ctx = ExitStack()
    sing = ctx.enter_context(tc.tile_pool(name="sing", bufs=1))
    wstream = ctx.enter_context(tc.tile_pool(name="wstream", bufs=2))
    evj = ctx.enter_context(tc.tile_pool(name="evj", bufs=6))
    tbufs = ctx.enter_context(tc.tile_pool(name="tbufs", bufs=6))
    stats = ctx.enter_context(tc.tile_pool(name="stats", bufs=1))
    final = ctx.enter_context(tc.tile_pool(name="final", bufs=2))

    # ---- persistent tiles ----
    s_xT = sing.tile([128, 4, NPOS], BF16)          # xT Ktiles
    nc.sync.dma_start(out=s_xT, in_=xT.rearrange("(kt p) n -> p kt n", p=128))
    s_bqk = sing.tile([1, 2 * E * H], BF16)
    nc.sync.dma_start(out=s_bqk, in_=bqk[:, :])
    s_bv = sing.tile([1, E * H], BF16)
    nc.sync.dma_start(out=s_bv, in_=bv[:, :])
    s_bu = sing.tile([1, E], BF16)
    nc.sync.dma_start(out=s_bu, in_=bu[:, :])
    s_ones = sing.tile([1, 512], BF16)
    nc.vector.memset(s_ones, 1.0)
    s_zero = sing.tile([128, 512], BF16)
    nc.vector.memset(s_zero, 0.0)

    q_pack = sing.tile([128, NPOS // 4, 512], BF16)   # rows 32g+h ; +8,+9 = -tau rows
    k_pack = sing.tile([128, NPOS // 4, 512], BF16)   # rows 32g+8/9 = ones
    nc.vector.memset(k_pack[:, :, :], 1.0)   # rows 32g+8/9 stay 1.0; k DMAs overwrite 0..7
    vT_pack = sing.tile([128, NPOS, 4, 8], BF16)      # [j_lo, pos, j_hi, h]
    resf = sing.tile([128, 8, 4, NPOS], BF16)         # res_flat^T [(i_lo),(h,i_hi,pos)]
    stg_ctx = tc.tile_pool(name="stg", bufs=1)
    stg = stg_ctx.__enter__()
    q_stage = stg.tile([NPOS, E * H], BF16)
    k_stage = stg.tile([NPOS, E * H], BF16)

    # ---- projections: q, k  (out[pos, hi] = xT.T @ W + b) ----
    pp_ctx = tc.tile_pool(name="pp", bufs=2, space="PSUM")
    pp = pp_ctx.__enter__()
    for c in range(16):
        wchunk = wstream.tile([128, 4, 512], BF16, tag="wqk")
        nc.sync.dma_start(
            out=wchunk,
            in_=wqk[:, 512 * c: 512 * (c + 1)].rearrange("(kt p) n -> p kt n", p=128))
        ps = pp.tile([NPOS, 512], F32, tag="psproj")
        for kt in range(4):
            nc.tensor.matmul(ps, s_xT[:, kt, :], wchunk[:, kt, :],
                             start=(kt == 0), stop=False)
        nc.tensor.matmul(ps, s_ones[:, :NPOS], s_bqk[:, 512 * c: 512 * (c + 1)],
                         start=False, stop=True)
        stage = q_stage if c < 8 else k_stage
        cc = c % 8
        nc.vector.tensor_scalar(stage[:, 512 * cc: 512 * (cc + 1)], ps, 1.0, None,
                                AL.mult, AL.bypass)
    # repack q/k into [h + 32*(p%4), p//4, e]
    for p in range(NPOS):
        g, pg = p % 4, p // 4
        nc.sync.dma_start(
            out=q_pack[32 * g: 32 * g + 8, pg, :],
            in_=q_stage[p: p + 1, :].rearrange("p (h e) -> p h e", h=8))
        nc.sync.dma_start(
            out=k_pack[32 * g: 32 * g + 8, pg, :],
            in_=k_stage[p: p + 1, :].rearrange("p (h e) -> p h e", h=8))

    stg_ctx.__exit__(None, None, None)

    # ---- projection v (transposed orientation): out[hj, pos] = Wv.T @ x ----
    for c in range(8):
        wvchunk = wstream.tile([128, 4, 512], BF16, tag="wv")
        nc.sync.dma_start(
            out=wvchunk,
            in_=wv[:, 512 * c: 512 * (c + 1)].rearrange("(kt p) n -> p kt n", p=128))
        for sub in range(4):
            t = 4 * c + sub          # hj-tile index: h = t // 4, j_hi = t % 4
            h, j_hi = t // 4, t % 4
            psv = pp.tile([128, NPOS], F32, tag="psv")
            for kt in range(4):
                nc.tensor.matmul(psv, wvchunk[:, kt, 128 * sub: 128 * (sub + 1)],
                                 s_xT[:, kt, :], start=(kt == 0), stop=False)
            nc.tensor.matmul(psv, s_bv[:, 128 * t: 128 * (t + 1)],
                             s_ones[:, :NPOS], start=False, stop=True)
            nc.vector.tensor_scalar(vT_pack[:, :, j_hi, h], psv, 1.0, None,
                                    AL.mult, AL.bypass)
    pp_ctx.__exit__(None, None, None)

    # ---- per-round processing ----
    pd_ctx = tc.tile_pool(name="pd", bufs=4, space="PSUM")
    pd = pd_ctx.__enter__()
    pf_ctx = tc.tile_pool(name="pf", bufs=1, space="PSUM")
    pf = pf_ctx.__enter__()
    pt_ctx = tc.tile_pool(name="ptp", bufs=2, space="PSUM")
    ptp = pt_ctx.__enter__()
    for r in range(NROUND):
        p0 = r * RPOS
        Xa = stats.tile([128, NT_R, 512], BF16, tag="xa")
        musum = stats.tile([128, NT_R], F32, tag="musum")
        mxh = stats.tile([128, NT_R], F32, tag="mxh")
        sqq = stats.tile([128, NT_R], F32, tag="sqq")

        # stage B: dot + evict + stats
        for pl in range(RPOS):
            p = p0 + pl
            g, pg = p % 4, p // 4
            for it in range(4):
                t = pl * 4 + it
                psd = pd.tile([128, 512], F32, tag="psd")
                nc.tensor.matmul(psd, q_pack[32 * g: 32 * g + 8, pg,
                                             128 * it: 128 * (it + 1)],
                                 k_pack[32 * g: 32 * g + 8, pg, :],
                                 start=True, stop=True,
                                 tile_position=(32 * g, 0))
                nc.scalar.activation(Xa[:, t, :], psd, AF.Copy, bias=0.0,
                                     scale=1.0, accum_out=musum[:, t: t + 1])
                ej = evj.tile([128, 256], BF16, tag="ej")
                nc.vector.tensor_scalar(ej, psd[:, 0:256], 1.0, None,
                                        AL.mult, AL.max,
                                        accum_out=mxh[:, t: t + 1])
        # sumsq (quarter width) from evicted bf16 scores
        for t in range(NT_R):
            ej2 = evj.tile([128, 128], BF16, tag="ej2")
            nc.vector.scalar_tensor_tensor(ej2, Xa[:, t, 0:128], 1.0,
                                           Xa[:, t, 0:128], AL.bypass, AL.mult,
                                           accum_out=sqq[:, t: t + 1])

        # ---- seed ----
        mu = stats.tile([128, NT_R], F32, tag="mu")
        nc.vector.tensor_scalar(mu, musum, 1.0 / 512.0, None, AL.mult, AL.bypass)
        dmx = stats.tile([128, NT_R], F32, tag="dmx")
        nc.vector.tensor_tensor(dmx, mxh, mu, AL.subtract)
        var = stats.tile([128, NT_R], F32, tag="var")
        # var = sqq/128 - mu^2   (clamped)
        nc.vector.scalar_tensor_tensor(var, mu, 0.0, mu, AL.mult, AL.mult)  # mu^2
        nc.vector.scalar_tensor_tensor(var, sqq, 1.0 / 128.0, var,
                                       AL.mult, AL.subtract)
        nc.vector.tensor_scalar(var, var, 1e-8, None, AL.max, AL.bypass)
        sg = stats.tile([128, NT_R], F32, tag="sg")
        nc.scalar.activation(sg, var, AF.Sqrt, bias=0.0, scale=1.0)
        # pred = c0+margin + c1*dmx + c2*sg + c3*dmx^2 + c4*sg*dmx
        c0, c1, c2, c3, c4 = SEED_COEF
        pred = stats.tile([128, NT_R], F32, tag="pred")
        tmp = stats.tile([128, NT_R], F32, tag="tmp")
        nc.vector.tensor_scalar(pred, dmx, c1, c0 + SEED_MARGIN, AL.mult, AL.add)
        nc.vector.tensor_scalar(tmp, sg, c2, None, AL.mult, AL.bypass)
        nc.vector.tensor_tensor(pred, pred, tmp, AL.add)
        nc.vector.scalar_tensor_tensor(tmp, dmx, c3, dmx, AL.mult, AL.mult)
        nc.vector.tensor_tensor(pred, pred, tmp, AL.add)
        nc.vector.scalar_tensor_tensor(tmp, sg, c4, dmx, AL.mult, AL.mult)
        nc.vector.tensor_tensor(pred, pred, tmp, AL.add)
        # tau = max(mxh - pred, mxh - 1)
        tau = stats.tile([128, NT_R], F32, tag="tau")
        nc.vector.tensor_tensor(tau, mxh, pred, AL.subtract)
        nc.vector.tensor_scalar(tmp, mxh, -1.0, None, AL.add, AL.bypass)
        nc.vector.tensor_tensor(tau, tau, tmp, AL.max)
        ntau = stats.tile([128, NT_R], F32, tag="ntau")
        S1 = stats.tile([128, NT_R], F32, tag="S1")
        S2 = stats.tile([128, NT_R], F32, tag="S2")
        dlt = stats.tile([128, NT_R], F32, tag="dlt")

        # ---- 2 Newton iterations ----
        for itn in range(2):
            half = HALF1 and itn == 0
            W = 256 if half else 512
            tgt = -0.5 if half else -1.0
            nc.vector.tensor_scalar(ntau, tau, -1.0, None, AL.mult, AL.bypass)
            for t in range(NT_R):
                tb = tbufs.tile([128, 512], BF16, tag="tb")
                if t % ACT_A_EVERY == 0:
                    nc.scalar.activation(tb[:, :W], Xa[:, t, :W], AF.Relu,
                                         bias=ntau[:, t: t + 1], scale=1.0,
                                         accum_out=S1[:, t: t + 1])
                else:
                    nc.vector.scalar_tensor_tensor(tb[:, :W], Xa[:, t, :W],
                                                   tau[:, t: t + 1],
                                                   s_zero[:, :W],
                                                   AL.subtract, AL.max,
                                                   accum_out=S1[:, t: t + 1])
                ej3 = evj.tile([128, 512], BF16, tag="ej3")
                if t % 8 < B_DVE_OF8:
                    nc.vector.scalar_tensor_tensor(ej3[:, :W], tb[:, :W], 1.0,
                                                   tb[:, :W],
                                                   AL.bypass, AL.mult,
                                                   accum_out=S2[:, t: t + 1])
                else:
                    nc.scalar.activation(ej3[:, :W], tb[:, :W], AF.Square,
                                         bias=0.0, scale=1.0,
                                         accum_out=S2[:, t: t + 1])
            # delta = (S2 + tgt) * 0.5 / S1 ;  tau += delta
            rec = stats.tile([128, NT_R], F32, tag="rec")
            nc.vector.reciprocal(rec, S1)
            nc.vector.tensor_scalar(dlt, S2, tgt, 0.5, AL.add, AL.mult)
            nc.vector.tensor_tensor(dlt, dlt, rec, AL.mult)
            nc.vector.tensor_tensor(tau, tau, dlt, AL.add)

        # Z = S2 - 2*delta*S1 ; Zinv
        Z = stats.tile([128, NT_R], F32, tag="Z")
        nc.vector.tensor_tensor(Z, dlt, S1, AL.mult)
        nc.vector.scalar_tensor_tensor(Z, Z, -2.0, S2, AL.mult, AL.add)
        nc.vector.tensor_scalar(Z, Z, 1e-30, None, AL.max, AL.bypass)
        Zinv = stats.tile([128, NT_R], F32, tag="Zinv")
        nc.vector.reciprocal(Zinv, Z)

        # ---- tau split (bf16 hi + residual) + transpose into q_pack rows ----
        nc.vector.tensor_scalar(ntau, tau, -1.0, None, AL.mult, AL.bypass)
        nth_b = stats.tile([128, NT_R], BF16, tag="nth_b")
        nc.vector.tensor_scalar(nth_b, ntau, 1.0, None, AL.mult, AL.bypass)
        nth_f = stats.tile([128, NT_R], F32, tag="nth_f")
        nc.vector.tensor_scalar(nth_f, nth_b, 1.0, None, AL.mult, AL.bypass)
        nr_b = stats.tile([128, NT_R], BF16, tag="nr_b")
        nc.vector.tensor_tensor(ntau, ntau, nth_f, AL.subtract)   # residual (f32)
        nc.vector.tensor_scalar(nr_b, ntau, 1.0, None, AL.mult, AL.bypass)
        tauI = stats.tile([128, 8 * RPOS], BF16, tag="tauI")
        nc.vector.tensor_scalar(
            tauI.rearrange("q (p it c) -> q p it c", it=4, c=2)[:, :, :, 0],
            nth_b.rearrange("q (p it) -> q p it", it=4), 1.0, None,
            AL.mult, AL.bypass)
        nc.vector.tensor_scalar(
            tauI.rearrange("q (p it c) -> q p it c", it=4, c=2)[:, :, :, 1],
            nr_b.rearrange("q (p it) -> q p it", it=4), 1.0, None,
            AL.mult, AL.bypass)
        tauIT = stats.tile([128, 8 * RPOS], BF16, tag="tauIT")
        for blk in range(RPOS // 16):
            nc.sync.dma_start(out=tauIT[:, 128 * blk: 128 * (blk + 1)],
                              in_=tauI[:, 128 * blk: 128 * (blk + 1)],
                              transpose=True)
        tauIT_v = tauIT.rearrange("q (blk f) -> q blk f", f=128)
        for pl in range(RPOS):
            p = p0 + pl
            g, pg = p % 4, p // 4
            blk, row = pl // 16, (pl % 16) * 8
            nc.sync.dma_start(
                out=q_pack[32 * g + 8: 32 * g + 9, pg, :]
                    .rearrange("p (a b) -> p a b", a=4),
                in_=tauIT_v[row: row + 8: 2, blk, :])
            nc.sync.dma_start(
                out=q_pack[32 * g + 9: 32 * g + 10, pg, :]
                    .rearrange("p (a b) -> p a b", a=4),
                in_=tauIT_v[row + 1: row + 8: 2, blk, :])

        # ---- final phase: dot^T - tau, p^T, res^T, normalize+scatter ----
        for q4 in range(RPOS // 4):        # groups of 4 positions
            psr = pf.tile([128, 512], F32, tag="psr")
            pTq = [None] * 4
            for gl in range(4):
                pl = q4 * 4 + gl
                p = p0 + pl
                g, pg = p % 4, p // 4
                pT = final.tile([128, 4, 512], BF16, tag="pt")
                pTq[gl] = pT
                for jt in range(4):
                    psT = ptp.tile([128, 512], F32, tag="pst")
                    nc.tensor.matmul(psT,
                                     k_pack[32 * g: 32 * g + 10, pg,
                                            128 * jt: 128 * (jt + 1)],
                                     q_pack[32 * g: 32 * g + 10, pg, :],
                                     start=True, stop=True,
                                     tile_position=(32 * g, 0))
                    tb2 = tbufs.tile([128, 512], BF16, tag="tb2")
                    nc.scalar.activation(tb2, psT, AF.Relu, bias=0.0,
                                         scale=1.0)
                    if jt % 2 == 0:
                        nc.vector.scalar_tensor_tensor(pT[:, jt, :], tb2, 1.0,
                                                       tb2, AL.bypass, AL.mult)
                    else:
                        nc.scalar.activation(pT[:, jt, :], tb2, AF.Square,
                                             bias=0.0, scale=1.0)
                # res^T for this position into col-group 32*gl of psr
                for jt in range(4):
                    nc.tensor.matmul(psr[32 * gl: 32 * gl + 8, :],
                                     vT_pack[:, p, jt, :], pTq[gl][:, jt, :],
                                     start=(jt == 0), stop=(jt == 3),
                                     tile_position=(0, 32 * gl))
            resN = final.tile([128, 512], BF16, tag="resn")
            nc.scalar.activation(resN, psr, AF.Copy, bias=0.0, scale=1.0)
            for i_hi in range(4):
                rT = final.tile([128, 128], BF16, tag="rt")
                nc.sync.dma_start(out=rT, in_=resN[:, 128 * i_hi: 128 * (i_hi + 1)],
                                  transpose=True)
                for gl in range(4):
                    pl = q4 * 4 + gl
                    p = p0 + pl
                    t = pl * 4 + i_hi
                    dst = resf[:, :, i_hi, p]       # [128, 8] strided (h)
                    if gl % 2 == 0:
                        nc.scalar.activation(dst, rT[:, 32 * gl: 32 * gl + 8],
                                             AF.Copy, bias=0.0,
                                             scale=Zinv[:, t: t + 1])
                    else:
                        nc.vector.tensor_scalar(dst, rT[:, 32 * gl: 32 * gl + 8],
                                                Zinv[:, t: t + 1], None,
                                                AL.mult, AL.bypass)

    pt_ctx.__exit__(None, None, None)
    pf_ctx.__exit__(None, None, None)
    pd_ctx.__exit__(None, None, None)

    # ---- output projection ----
    po_ctx = tc.tile_pool(name="po", bufs=1, space="PSUM")
    po = po_ctx.__enter__()
    pso = po.tile([NPOS, 512], F32, tag="pso")
    for t in range(32):
        h, i_hi = t // 4, t % 4
        wuk = wstream.tile([128, 512], BF16, tag="wu")
        nc.sync.dma_start(out=wuk, in_=wu[128 * t: 128 * (t + 1), :])
        nc.tensor.matmul(pso, resf[:, h, i_hi, :], wuk,
                         start=(t == 0), stop=False)
    nc.tensor.matmul(pso, s_ones[:, :NPOS], s_bu, start=False, stop=True)
    out_sb = sing.tile([NPOS, 512], F32)
    nc.scalar.activation(out_sb, pso, AF.Copy, bias=0.0, scale=1.0)
    nc.sync.dma_start(out=out[:, :], in_=out_sb)
    po_ctx.__exit__(None, None, None)
    ctx.close()


_NC_CACHE = {}


def _get_nc():
    if "nc" not in _NC_CACHE:
        _NC_CACHE["nc"] = build_nc()
    return _NC_CACHE["nc"]


def kernel(x, Wq, bq, Wk, bk, Wv, bv, Wu, bu, alpha):
    alpha = float(np.asarray(alpha))
    assert abs(alpha - 1.5) < 1e-6, "kernel specialized for alpha=1.5"
    s = np.float32((alpha - 1.0) / np.sqrt(E))
    bf = ml_dtypes.bfloat16
    x2 = np.asarray(x, np.float32).reshape(P_TOT, E)
    wqk_h = np.concatenate([np.asarray(Wq, np.float32),
                            np.asarray(Wk, np.float32) * s], axis=1).astype(bf)
    bqk_h = np.concatenate([np.asarray(bq, np.float32),
                            np.asarray(bk, np.float32) * s])[None, :].astype(bf)
    wv_h = np.asarray(Wv, np.float32).astype(bf)
    bv_h = np.asarray(bv, np.float32)[None, :].astype(bf)
    wu_h = np.asarray(Wu, np.float32).astype(bf)
    bu_h = np.asarray(bu, np.float32)[None, :].astype(bf)

    nc = _get_nc()
    in_maps = []
    for core in range(N_CORES):
        sh = x2[core * NPOS:(core + 1) * NPOS, :]     # [NPOS, E]
        in_maps.append(dict(
            xT=np.ascontiguousarray(sh.T).astype(bf),
            wqk=wqk_h, bqk=bqk_h, wv=wv_h, bv=bv_h, wu=wu_h, bu=bu_h,
        ))
    r = run_bass_kernel_spmd(nc, in_maps, core_ids=list(range(N_CORES)))
    outs = [r.results[c]["out"] for c in range(N_CORES)]
    full = np.concatenate(outs, axis=0).reshape(B, C, E).astype(np.float32)
    return full


# revision 11
# speedup vs baseline: 1.1472x; 1.0021x over previous
"""AlphaEntmax sparse-attention kernel for 8 Trainium2 NeuronCores.

Strategy (data-parallel over the 512 (b,c) positions, 64 per core):
  - projections q,k,v on PE (bf16, scale (alpha-1)/sqrt(E) folded into Wk)
  - per position: dot[i,j] = sum_h q[h,i] k[h,j] on PE (K=8 matmuls),
    evicted PSUM->SBUF bf16 by ACT (with row-sum accum) while DVE extracts
    row max / sumsq stats for the tau seed
  - entmax-1.5 threshold tau solved with a calibrated seed + 2 Newton
    iterations on the bf16 scores (scalar_tensor_tensor relu/square passes
    with fused per-row accumulation, split across ACT and DVE)
  - final p computed transposed: PE recomputes dot^T with two extra
    contraction rows carrying -tau (bf16 hi+lo split) so relu^2 of the PSUM
    gives p^T directly; res^T = v^T @ p^T on PE with 4-position col-grouping
  - normalization by Z = sum p (predicted from the last Newton accums) is
    folded into the res_flat^T scatter copies; final Wu matmul on PE.
"""
import numpy as np
import ml_dtypes

import concourse.bass as bass
import concourse.tile as tile
from concourse import mybir
from concourse.bass_utils import run_bass_kernel_spmd

F32 = mybir.dt.float32
BF16 = mybir.dt.bfloat16
AL = mybir.AluOpType
AF = mybir.ActivationFunctionType

N_CORES = 8
B, C, E, H = 2, 256, 512, 8
P_TOT = B * C                  # 512 positions
NPOS = P_TOT // N_CORES        # 64 per core
NROUND = 4                     # positions processed in rounds (SBUF budget)
RPOS = NPOS // NROUND          # 16 positions per round
NT_R = RPOS * 4                # score tiles per round (4 i-tiles per position)

# tau-seed calibration (fit offline on the model's input distribution):
# gap_half = mxh - tau* ~= c0 + c1*dmx + c2*sg + c3*dmx^2 + c4*sg*dmx
SEED_COEF = (0.063100, 0.970412, -0.742733, 0.211108, -1.770962)
SEED_MARGIN = 0.01

# Newton-pass engine splits (tunable): pass A on ACT every ACT_A_EVERY-th
# tile (ACT relu+accum gives t and S1 in one op; DVE path uses two 4x
# tensor_scalar ops).  pass B: DVE scalar_tensor_tensor+accum for tiles with
# (t % 8) < B_DVE_OF8, ACT Square+accum for the rest.
ACT_A_EVERY = 8
B_DVE_OF8 = 5
HALF1 = True


def _split_waits(nc, limit=1):
    """walrus here only supports one sync-wait per instruction: split extras
    into chained same-engine drains."""
    n = 0
    for fn in nc.m.functions:
        for bb in fn.blocks:
            newlist = []
            for ins in bb.instructions:
                si = getattr(ins, "sync_info", None)
                if si is not None and len(si.on_wait) > limit:
                    waits = list(si.on_wait)
                    while len(waits) > limit:
                        chunk, waits = waits[:limit], waits[limit:]
                        d = mybir.InstDrain(name=f"Wsplit-{n}")
                        n += 1
                        d.engine = ins.engine
                        d.sync_info = mybir.SyncInfo(on_wait=list(chunk), on_update=[])
                        newlist.append(d)
                    del si.on_wait[:]
                    si.on_wait.extend(waits)
                newlist.append(ins)
            del bb.instructions[:]
            for x in newlist:
                bb.add_instruction(x)
    return n


def build_nc():
    nc = bass.Bass("TRN2", target_bir_lowering=False, debug=False)

    xT = nc.declare_dram_parameter("xT", [E, NPOS], BF16, isOutput=False)
    wqk = nc.declare_dram_parameter("wqk", [E, 2 * E * H], BF16, isOutput=False)
    bqk = nc.declare_dram_parameter("bqk", [1, 2 * E * H], BF16, isOutput=False)
    wv = nc.declare_dram_parameter("wv", [E, E * H], BF16, isOutput=False)
    bv = nc.declare_dram_parameter("bv", [1, E * H], BF16, isOutput=False)
    wu = nc.declare_dram_parameter("wu", [E * H, E], BF16, isOutput=False)
    bu = nc.declare_dram_parameter("bu", [1, E], BF16, isOutput=False)
    out = nc.declare_dram_parameter("out", [NPOS, E], F32, isOutput=True)

    with tile.TileContext(nc) as tc:
        _build_body(nc, tc, xT, wqk, bqk, wv, bv, wu, bu, out)
    _split_waits(nc)
    return nc


def _build_body(nc, tc, xT, wqk, bqk, wv, bv, wu, bu, out):
    from contextlib import ExitStack
    ctx = ExitStack()
    sing = ctx.enter_context(tc.tile_pool(name="sing", bufs=1))
    wstream = ctx.enter_context(tc.tile_pool(name="wstream", bufs=2))
    evj = ctx.enter_context(tc.tile_pool(name="evj", bufs=6))
    tbufs = ctx.enter_context(tc.tile_pool(name="tbufs", bufs=6))
    stats = ctx.enter_context(tc.tile_pool(name="stats", bufs=1))
    final = ctx.enter_context(tc.tile_pool(name="final", bufs=2))

    # ---- persistent tiles ----
    s_xT = sing.tile([128, 4, NPOS], BF16)          # xT Ktiles
    nc.sync.dma_start(out=s_xT, in_=xT.rearrange("(kt p) n -> p kt n", p=128))
    s_bqk = sing.tile([1, 2 * E * H], BF16)
    nc.sync.dma_start(out=s_bqk, in_=bqk[:, :])
    s_bv = sing.tile([1, E * H], BF16)
    nc.sync.dma_start(out=s_bv, in_=bv[:, :])
    s_bu = sing.tile([1, E], BF16)
    nc.sync.dma_start(out=s_bu, in_=bu[:, :])
    s_ones = sing.tile([1, 512], BF16)
    nc.vector.memset(s_ones, 1.0)
    s_zero = sing.tile([128, 512], BF16)
    nc.vector.memset(s_zero, 0.0)

    q_pack = sing.tile([128, NPOS // 4, 512], BF16)   # rows 32g+h ; +8,+9 = -tau rows
    k_pack = sing.tile([128, NPOS // 4, 512], BF16)   # rows 32g+8/9 = ones
    nc.vector.memset(k_pack[:, :, :], 1.0)   # rows 32g+8/9 stay 1.0; k DMAs overwrite 0..7
    vT_pack = sing.tile([128, NPOS, 4, 8], BF16)      # [j_lo, pos, j_hi, h]
    resf = sing.tile([128, 8, 4, NPOS], BF16)         # res_flat^T [(i_lo),(h,i_hi,pos)]
    stg_ctx = tc.tile_pool(name="stg", bufs=1)
    stg = stg_ctx.__enter__()
    q_stage = stg.tile([NPOS, E * H], BF16)
    k_stage = stg.tile([NPOS, E * H], BF16)

    # ---- projections: q, k  (out[pos, hi] = xT.T @ W + b) ----
    pp_ctx = tc.tile_pool(name="pp", bufs=2, space="PSUM")
    pp = pp_ctx.__enter__()
    for c in range(16):
        wchunk = wstream.tile([128, 4, 512], BF16, tag="wqk")
        nc.sync.dma_start(
            out=wchunk,
            in_=wqk[:, 512 * c: 512 * (c + 1)].rearrange("(kt p) n -> p kt n", p=128))
        ps = pp.tile([NPOS, 512], F32, tag="psproj")
        for kt in range(4):
            nc.tensor.matmul(ps, s_xT[:, kt, :], wchunk[:, kt, :],
                             start=(kt == 0), stop=False)
        nc.tensor.matmul(ps, s_ones[:, :NPOS], s_bqk[:, 512 * c: 512 * (c + 1)],
                         start=False, stop=True)
        stage = q_stage if c < 8 else k_stage
        cc = c % 8
        nc.vector.tensor_scalar(stage[:, 512 * cc: 512 * (cc + 1)], ps, 1.0, None,
                                AL.mult, AL.bypass)
    # repack q/k into [h + 32*(p%4), p//4, e]
    for p in range(NPOS):
        g, pg = p % 4, p // 4
        nc.sync.dma_start(
            out=q_pack[32 * g: 32 * g + 8, pg, :],
            in_=q_stage[p: p + 1, :].rearrange("p (h e) -> p h e", h=8))
        nc.sync.dma_start(
            out=k_pack[32 * g: 32 * g + 8, pg, :],
            in_=k_stage[p: p + 1, :].rearrange("p (h e) -> p h e", h=8))

    stg_ctx.__exit__(None, None, None)

    # ---- projection v (transposed orientation): out[hj, pos] = Wv.T @ x ----
    for c in range(8):
        wvchunk = wstream.tile([128, 4, 512], BF16, tag="wv")
        nc.sync.dma_start(
            out=wvchunk,
            in_=wv[:, 512 * c: 512 * (c + 1)].rearrange("(kt p) n -> p kt n", p=128))
        for sub in range(4):
            t = 4 * c + sub          # hj-tile index: h = t // 4, j_hi = t % 4
            h, j_hi = t // 4, t % 4
            psv = pp.tile([128, NPOS], F32, tag="psv")
            for kt in range(4):
                nc.tensor.matmul(psv, wvchunk[:, kt, 128 * sub: 128 * (sub + 1)],
                                 s_xT[:, kt, :], start=(kt == 0), stop=False)
            nc.tensor.matmul(psv, s_bv[:, 128 * t: 128 * (t + 1)],
                             s_ones[:, :NPOS], start=False, stop=True)
            nc.vector.tensor_scalar(vT_pack[:, :, j_hi, h], psv, 1.0, None,
                                    AL.mult, AL.bypass)
    pp_ctx.__exit__(None, None, None)

    # ---- per-round processing ----
    pd_ctx = tc.tile_pool(name="pd", bufs=4, space="PSUM")
    pd = pd_ctx.__enter__()
    pf_ctx = tc.tile_pool(name="pf", bufs=1, space="PSUM")
    pf = pf_ctx.__enter__()
    pt_ctx = tc.tile_pool(name="ptp", bufs=2, space="PSUM")
    ptp = pt_ctx.__enter__()
    def emit_stageB_pos(r, pl, Xa, musum, mxh):
        p = r * RPOS + pl
        g, pg = p % 4, p // 4
        for it in range(4):
            t = pl * 4 + it
            psd = pd.tile([128, 512], F32, tag="psd")
            nc.tensor.matmul(psd, q_pack[32 * g: 32 * g + 8, pg,
                                         128 * it: 128 * (it + 1)],
                             k_pack[32 * g: 32 * g + 8, pg, :],
                             start=True, stop=True,
                             tile_position=(32 * g, 0))
            nc.scalar.activation(Xa[:, t, :], psd, AF.Copy, bias=0.0,
                                 scale=1.0, accum_out=musum[:, t: t + 1])
            ej = evj.tile([128, 256], BF16, tag="ej")
            nc.vector.tensor_scalar(ej, psd[:, 0:256], 1.0, None,
                                    AL.mult, AL.max,
                                    accum_out=mxh[:, t: t + 1])

    def emit_final_group(r, q4, Zinv):
        p0 = r * RPOS
        psr = pf.tile([128, 512], F32, tag="psr")
        for gl in range(4):
            pl = q4 * 4 + gl
            p = p0 + pl
            g, pg = p % 4, p // 4
            pT = final.tile([128, 4, 512], BF16, tag="pt")
            for jt in range(4):
                psT = ptp.tile([128, 512], F32, tag="pst")
                nc.tensor.matmul(psT,
                                 k_pack[32 * g: 32 * g + 10, pg,
                                        128 * jt: 128 * (jt + 1)],
                                 q_pack[32 * g: 32 * g + 10, pg, :],
                                 start=True, stop=True,
                                 tile_position=(32 * g, 0))
                tb2 = tbufs.tile([128, 512], BF16, tag="tb2")
                nc.scalar.activation(tb2, psT, AF.Relu, bias=0.0,
                                     scale=1.0)
                if jt % 2 == 0:
                    nc.vector.scalar_tensor_tensor(pT[:, jt, :], tb2, 1.0,
                                                   tb2, AL.bypass, AL.mult)
                else:
                    nc.scalar.activation(pT[:, jt, :], tb2, AF.Square,
                                         bias=0.0, scale=1.0)
            for jt in range(4):
                nc.tensor.matmul(psr[32 * gl: 32 * gl + 8, :],
                                 vT_pack[:, p, jt, :], pT[:, jt, :],
                                 start=(jt == 0), stop=(jt == 3),
                                 tile_position=(0, 32 * gl))
        resN = final.tile([128, 512], BF16, tag="resn")
        nc.scalar.activation(resN, psr, AF.Copy, bias=0.0, scale=1.0)
        for i_hi in range(4):
            rT = final.tile([128, 128], BF16, tag="rt")
            nc.sync.dma_start(out=rT, in_=resN[:, 128 * i_hi: 128 * (i_hi + 1)],
                              transpose=True)
            for gl in range(4):
                pl = q4 * 4 + gl
                p = p0 + pl
                t = pl * 4 + i_hi
                dst = resf[:, :, i_hi, p]
                if gl % 2 == 0:
                    nc.scalar.activation(dst, rT[:, 32 * gl: 32 * gl + 8],
                                         AF.Copy, bias=0.0,
                                         scale=Zinv[:, t: t + 1])
                else:
                    nc.vector.tensor_scalar(dst, rT[:, 32 * gl: 32 * gl + 8],
                                            Zinv[:, t: t + 1], None,
                                            AL.mult, AL.bypass)

    prev_final = [None]          # (r, Zinv) pending
    for r in range(NROUND):
        p0 = r * RPOS
        Xa = stats.tile([128, NT_R, 512], BF16, tag="xa")
        musum = stats.tile([128, NT_R], F32, tag="musum")
        mxh = stats.tile([128, NT_R], F32, tag="mxh")
        sqq = stats.tile([128, NT_R], F32, tag="sqq")

        # stage B interleaved with previous round's final phase
        for pl in range(RPOS):
            emit_stageB_pos(r, pl, Xa, musum, mxh)
            if prev_final[0] is not None and pl % 4 == 3:
                pr, pZinv = prev_final[0]
                emit_final_group(pr, pl // 4, pZinv)
        # sumsq (quarter width) from evicted bf16 scores
        for t in range(NT_R):
            ej2 = evj.tile([128, 128], BF16, tag="ej2")
            nc.vector.scalar_tensor_tensor(ej2, Xa[:, t, 0:128], 1.0,
                                           Xa[:, t, 0:128], AL.bypass, AL.mult,
                                           accum_out=sqq[:, t: t + 1])

        # ---- seed ----
        mu = stats.tile([128, NT_R], F32, tag="mu")
        nc.vector.tensor_scalar(mu, musum, 1.0 / 512.0, None, AL.mult, AL.bypass)
        dmx = stats.tile([128, NT_R], F32, tag="dmx")
        nc.vector.tensor_tensor(dmx, mxh, mu, AL.subtract)
        var = stats.tile([128, NT_R], F32, tag="var")
        # var = sqq/128 - mu^2   (clamped)
        nc.vector.scalar_tensor_tensor(var, mu, 0.0, mu, AL.mult, AL.mult)  # mu^2
        nc.vector.scalar_tensor_tensor(var, sqq, 1.0 / 128.0, var,
                                       AL.mult, AL.subtract)
        nc.vector.tensor_scalar(var, var, 1e-8, None, AL.max, AL.bypass)
        sg = stats.tile([128, NT_R], F32, tag="sg")
        nc.scalar.activation(sg, var, AF.Sqrt, bias=0.0, scale=1.0)
        # pred = c0+margin + c1*dmx + c2*sg + c3*dmx^2 + c4*sg*dmx
        c0, c1, c2, c3, c4 = SEED_COEF
        pred = stats.tile([128, NT_R], F32, tag="pred")
        tmp = stats.tile([128, NT_R], F32, tag="tmp")
        nc.vector.tensor_scalar(pred, dmx, c1, c0 + SEED_MARGIN, AL.mult, AL.add)
        nc.vector.tensor_scalar(tmp, sg, c2, None, AL.mult, AL.bypass)
        nc.vector.tensor_tensor(pred, pred, tmp, AL.add)
        nc.vector.scalar_tensor_tensor(tmp, dmx, c3, dmx, AL.mult, AL.mult)
        nc.vector.tensor_tensor(pred, pred, tmp, AL.add)
        nc.vector.scalar_tensor_tensor(tmp, sg, c4, dmx, AL.mult, AL.mult)
        nc.vector.tensor_tensor(pred, pred, tmp, AL.add)
        # tau = max(mxh - pred, mxh - 1)
        tau = stats.tile([128, NT_R], F32, tag="tau")
        nc.vector.tensor_tensor(tau, mxh, pred, AL.subtract)
        nc.vector.tensor_scalar(tmp, mxh, -1.0, None, AL.add, AL.bypass)
        nc.vector.tensor_tensor(tau, tau, tmp, AL.max)
        ntau = stats.tile([128, NT_R], F32, tag="ntau")
        S1 = stats.tile([128, NT_R], F32, tag="S1")
        S2 = stats.tile([128, NT_R], F32, tag="S2")
        dlt = stats.tile([128, NT_R], F32, tag="dlt")

        # ---- 2 Newton iterations ----
        for itn in range(2):
            half = HALF1 and itn == 0
            W = 256 if half else 512
            tgt = -0.5 if half else -1.0
            nc.vector.tensor_scalar(ntau, tau, -1.0, None, AL.mult, AL.bypass)
            for t in range(NT_R):
                tb = tbufs.tile([128, 512], BF16, tag="tb")
                if t % ACT_A_EVERY == 0:
                    nc.scalar.activation(tb[:, :W], Xa[:, t, :W], AF.Relu,
                                         bias=ntau[:, t: t + 1], scale=1.0,
                                         accum_out=S1[:, t: t + 1])
                else:
                    nc.vector.scalar_tensor_tensor(tb[:, :W], Xa[:, t, :W],
                                                   tau[:, t: t + 1],
                                                   s_zero[:, :W],
                                                   AL.subtract, AL.max,
                                                   accum_out=S1[:, t: t + 1])
                ej3 = evj.tile([128, 512], BF16, tag="ej3")
                if t % 8 < B_DVE_OF8:
                    nc.vector.scalar_tensor_tensor(ej3[:, :W], tb[:, :W], 1.0,
                                                   tb[:, :W],
                                                   AL.bypass, AL.mult,
                                                   accum_out=S2[:, t: t + 1])
                else:
                    nc.scalar.activation(ej3[:, :W], tb[:, :W], AF.Square,
                                         bias=0.0, scale=1.0,
                                         accum_out=S2[:, t: t + 1])
            # delta = (S2 + tgt) * 0.5 / S1 ;  tau += delta
            rec = stats.tile([128, NT_R], F32, tag="rec")
            nc.vector.reciprocal(rec, S1)
            nc.vector.tensor_scalar(dlt, S2, tgt, 0.5, AL.add, AL.mult)
            nc.vector.tensor_tensor(dlt, dlt, rec, AL.mult)
            nc.vector.tensor_tensor(tau, tau, dlt, AL.add)

        # Z = S2 - 2*delta*S1 ; Zinv
        Z = stats.tile([128, NT_R], F32, tag="Z")
        nc.vector.tensor_tensor(Z, dlt, S1, AL.mult)
        nc.vector.scalar_tensor_tensor(Z, Z, -2.0, S2, AL.mult, AL.add)
        nc.vector.tensor_scalar(Z, Z, 1e-30, None, AL.max, AL.bypass)
        Zinv = stats.tile([128, NT_R], F32, tag="Zinv")
        nc.vector.reciprocal(Zinv, Z)

        # ---- tau split (bf16 hi + residual) + transpose into q_pack rows ----
        nc.vector.tensor_scalar(ntau, tau, -1.0, None, AL.mult, AL.bypass)
        nth_b = stats.tile([128, NT_R], BF16, tag="nth_b")
        nc.vector.tensor_scalar(nth_b, ntau, 1.0, None, AL.mult, AL.bypass)
        nth_f = stats.tile([128, NT_R], F32, tag="nth_f")
        nc.vector.tensor_scalar(nth_f, nth_b, 1.0, None, AL.mult, AL.bypass)
        nr_b = stats.tile([128, NT_R], BF16, tag="nr_b")
        nc.vector.tensor_tensor(ntau, ntau, nth_f, AL.subtract)   # residual (f32)
        nc.vector.tensor_scalar(nr_b, ntau, 1.0, None, AL.mult, AL.bypass)
        tauI = stats.tile([128, 8 * RPOS], BF16, tag="tauI")
        nc.vector.tensor_scalar(
            tauI.rearrange("q (p it c) -> q p it c", it=4, c=2)[:, :, :, 0],
            nth_b.rearrange("q (p it) -> q p it", it=4), 1.0, None,
            AL.mult, AL.bypass)
        nc.vector.tensor_scalar(
            tauI.rearrange("q (p it c) -> q p it c", it=4, c=2)[:, :, :, 1],
            nr_b.rearrange("q (p it) -> q p it", it=4), 1.0, None,
            AL.mult, AL.bypass)
        tauIT = stats.tile([128, 8 * RPOS], BF16, tag="tauIT")
        for blk in range(RPOS // 16):
            nc.sync.dma_start(out=tauIT[:, 128 * blk: 128 * (blk + 1)],
                              in_=tauI[:, 128 * blk: 128 * (blk + 1)],
                              transpose=True)
        tauIT_v = tauIT.rearrange("q (blk f) -> q blk f", f=128)
        for pl in range(RPOS):
            p = p0 + pl
            g, pg = p % 4, p // 4
            blk, row = pl // 16, (pl % 16) * 8
            nc.sync.dma_start(
                out=q_pack[32 * g + 8: 32 * g + 9, pg, :]
                    .rearrange("p (a b) -> p a b", a=4),
                in_=tauIT_v[row: row + 8: 2, blk, :])
            nc.sync.dma_start(
                out=q_pack[32 * g + 9: 32 * g + 10, pg, :]
                    .rearrange("p (a b) -> p a b", a=4),
                in_=tauIT_v[row + 1: row + 8: 2, blk, :])

        # final phase deferred: interleaves with next round's stage B
        prev_final[0] = (r, Zinv)

    pr, pZinv = prev_final[0]
    for q4 in range(RPOS // 4):
        emit_final_group(pr, q4, pZinv)
    pt_ctx.__exit__(None, None, None)
    pf_ctx.__exit__(None, None, None)
    pd_ctx.__exit__(None, None, None)

    # ---- output projection ----
    po_ctx = tc.tile_pool(name="po", bufs=1, space="PSUM")
    po = po_ctx.__enter__()
    pso = po.tile([NPOS, 512], F32, tag="pso")
    for t in range(32):
        h, i_hi = t // 4, t % 4
        wuk = wstream.tile([128, 512], BF16, tag="wu")
        nc.sync.dma_start(out=wuk, in_=wu[128 * t: 128 * (t + 1), :])
        nc.tensor.matmul(pso, resf[:, h, i_hi, :], wuk,
                         start=(t == 0), stop=False)
    nc.tensor.matmul(pso, s_ones[:, :NPOS], s_bu, start=False, stop=True)
    out_sb = sing.tile([NPOS, 512], F32)
    nc.scalar.activation(out_sb, pso, AF.Copy, bias=0.0, scale=1.0)
    nc.sync.dma_start(out=out[:, :], in_=out_sb)
    po_ctx.__exit__(None, None, None)
    ctx.close()


_NC_CACHE = {}


def _get_nc():
    if "nc" not in _NC_CACHE:
        _NC_CACHE["nc"] = build_nc()
    return _NC_CACHE["nc"]


def kernel(x, Wq, bq, Wk, bk, Wv, bv, Wu, bu, alpha):
    alpha = float(np.asarray(alpha))
    assert abs(alpha - 1.5) < 1e-6, "kernel specialized for alpha=1.5"
    s = np.float32((alpha - 1.0) / np.sqrt(E))
    bf = ml_dtypes.bfloat16
    x2 = np.asarray(x, np.float32).reshape(P_TOT, E)
    wqk_h = np.concatenate([np.asarray(Wq, np.float32),
                            np.asarray(Wk, np.float32) * s], axis=1).astype(bf)
    bqk_h = np.concatenate([np.asarray(bq, np.float32),
                            np.asarray(bk, np.float32) * s])[None, :].astype(bf)
    wv_h = np.asarray(Wv, np.float32).astype(bf)
    bv_h = np.asarray(bv, np.float32)[None, :].astype(bf)
    wu_h = np.asarray(Wu, np.float32).astype(bf)
    bu_h = np.asarray(bu, np.float32)[None, :].astype(bf)

    nc = _get_nc()
    in_maps = []
    for core in range(N_CORES):
        sh = x2[core * NPOS:(core + 1) * NPOS, :]     # [NPOS, E]
        in_maps.append(dict(
            xT=np.ascontiguousarray(sh.T).astype(bf),
            wqk=wqk_h, bqk=bqk_h, wv=wv_h, bv=bv_h, wu=wu_h, bu=bu_h,
        ))
    r = run_bass_kernel_spmd(nc, in_maps, core_ids=list(range(N_CORES)))
    outs = [r.results[c]["out"] for c in range(N_CORES)]
    full = np.concatenate(outs, axis=0).reshape(B, C, E).astype(np.float32)
    return full
